# revision 1
# baseline (speedup 1.0000x reference)
"""Trainium2 Bass kernel for nn_AlignModel.

Computes out[b, j, i] = sigmoid(simp[b,j]·w_s + orig[b,i]·w_o + bias) where
orig/simp are the two halves of prop_state[b] ([B, 2S, D] -> [B,S,D] each),
w_o = W[0,:D], w_s = W[0,D:].

Sharding: data-parallel over batch B=8 across the 8 NeuronCores. Each core:
  in  x   [4096, 512] f32  (= prop_state[b])
  in  w   [1, 1024]   f32
  in  bvec[1, 1]      f32
  out out [2048, 2048] f32 (= sigmoid(s_s[:,None] + s_o[None,:] + b))

Structure (from ~20 NTFF-profile iterations; best measured 87.6 us):
  - Concurrent DMAs in a queue drain round-robin, so all transfers finish
    together at ~total/bandwidth.  The orig chunks therefore use GEOMETRIC
    sizes (1,1,2,4,4,4 tiles): early chunks surface quickly so the DVE
    multiply pipeline starts ~7 us sooner, while the aggregate stream
    still runs at full rate.  simp loads queue behind on the same Sync
    FIFO, WAW-gated (tiny DVE writes into their tiles) so they start only
    mid-phase-1a; output stores follow, keeping the DMA pipe continuously
    busy from first load to last store.
  - The orig half is consumed partition-outer (i = p*16 + n): contiguous
    per-partition input descriptors, and s_o[128,16] scatters straight
    into the broadcast row [1,2048] with tiny strided DMAs (no transpose),
    then one ScalarE cast to fp16.
  - so_row -> PSUM [128,2048] replication via rank-1 fp16 PE matmuls
    (fast even HAM-cold); the bias b is PSUM-seeded first and s_o
    accumulates on top.
  - Dot products: DVE tensor_mul + ScalarE Copy-with-accum (orig) / DVE
    tensor_reduce (simp); ScalarE is reserved for phase-2 sigmoids.
  - Each output row-tile is ONE ScalarE op
      out_t = Sigmoid(s_o_bcast + bias_col_t)   (PSUM -> SBUF).
    The first and last row-tiles ship as single 1 MiB stores (gated on one
    sigmoid, starting the drain earlier); the rest leave as 2 MiB pairs.
"""

import numpy as np

import concourse.mybir as mybir
from concourse import bacc, bass_utils
from concourse.tile import TileContext

P = 128          # partitions
D = 512          # feature dim
S = 2048         # sents
NT = S // P      # 16 tiles per half
OCHUNKS = [1, 1, 2, 4, 4, 4]   # orig tiles per chunk
SCH = 4          # simp tiles per chunk (1 MiB)
NSC = NT // SCH
NCORES = 8
F32 = mybir.dt.float32


def _kernel_body(tc, out, x, w, bvec):
    nc = tc.nc
    # orig half, partition-outer: i = p*NT + n
    xo_re = x[0:S, :].rearrange("(p n) d -> p n d", n=NT)
    # simp half, partition-inner: j = n*P + p  (bias needs column layout)
    xs_re = x[S:2 * S, :].rearrange("(n p) d -> p n d", p=P)

    with (
        tc.tile_pool(name="consts", bufs=1) as cpool,
        tc.tile_pool(name="xin", bufs=1) as xpool,
        tc.tile_pool(name="scratch", bufs=4) as spool,
        tc.tile_pool(name="outbuf", bufs=4) as opool,
        tc.tile_pool(name="psum", bufs=1, space="PSUM") as ppool,
    ):
        # --- orig input stream: geometric chunks, all in flight at once ---
        xo_tiles = []
        n0 = 0
        for c, sz in enumerate(OCHUNKS):
            xo = xpool.tile([P, sz, D], F32, tag=f"xo{c}", name=f"xo{c}")
            nc.sync.dma_start(out=xo, in_=xo_re[:, n0:n0 + sz, :])
            xo_tiles.append(xo)
            n0 += sz

        # simp tiles; their loads go on the Sync queue behind the orig
        # chunks but are gated by a tiny DVE write into each tile (WAW dep)
        # so the transfers only start once phase 1a is nearly done -- an
        # ungated DMA would be scheduled at t=0 and starve the orig stream.
        xs_tiles = [
            xpool.tile([P, SCH, D], F32, tag=f"xs{g}", name=f"xs{g}")
            for g in range(NSC)
        ]

        # w / b replicated across partitions by zero-stride DMA (SWDGE);
        # w_o first since it gates the first multiply.
        w_bc = cpool.tile([P, 2 * D], F32, tag="wbc")
        nc.gpsimd.dma_start(out=w_bc[:, 0:D],
                            in_=w[:, 0:D].broadcast_to([P, D]))
        nc.gpsimd.dma_start(out=w_bc[:, D:2 * D],
                            in_=w[:, D:2 * D].broadcast_to([P, D]))
        ones_row = cpool.tile([1, P], mybir.dt.float16, tag="ones")
        nc.gpsimd.memset(ones_row, 1.0)

        s_o_mat = cpool.tile([P, NT], F32, tag="somat")   # s_o[p*16+n] @ [p,n]
        s_sb_mat = cpool.tile([P, NT], F32, tag="ssmat")  # s_s + b, col t
        so_rowf = cpool.tile([1, S], F32, tag="sorowf")   # f32 scatter dest
        so_row = cpool.tile([1, S], mybir.dt.float16, tag="sorow")
        b_sb = cpool.tile([1, 1], F32, tag="bsb")
        nc.sync.dma_start(out=b_sb, in_=bvec)
        b_row = cpool.tile([1, 512], mybir.dt.float16, tag="brow")
        nc.gpsimd.memset(b_row, 0.0)
        nc.vector.tensor_scalar_add(b_row, b_row, b_sb)
        sob_psum = ppool.tile([P, S], F32, tag="sob")     # s_o on every row

        # --- phase 1a: orig half -> s_o -> so_row ---
        n0 = 0
        for c, sz in enumerate(OCHUNKS):
            xo = xo_tiles[c]
            for blk in range(sz):
                t = n0 + blk
                prod = spool.tile([P, D], F32, tag="prod", name=f"po{t}")
                nc.vector.tensor_mul(out=prod, in0=xo[:, blk, :],
                                     in1=w_bc[:, 0:D])
                nc.scalar.activation(
                    prod, prod, mybir.ActivationFunctionType.Copy,
                    accum_out=s_o_mat[:, t:t + 1])
                if t in (8, 10, 12):
                    gs = {8: (0,), 10: (1,), 12: (2, 3)}[t]
                    for g in gs:
                        nc.vector.tensor_copy(
                            out=xs_tiles[g][0:1, 0, 0:1],
                            in_=prod[0:1, 0:1])
            src = s_o_mat[:, n0:n0 + sz]
            dst = so_rowf.rearrange("o (p n) -> o p n", n=NT)[:, :, n0:n0 + sz]
            nc.scalar.dma_start(out=dst, in_=src)
            n0 += sz

        # single f32 -> fp16 cast on ScalarE (engine-local, no DMA semaphore
        # on the critical chain)
        nc.scalar.copy(so_row, so_rowf)

        # simp loads: queued on Sync behind the orig chunks, released by the
        # gate writes above
        for g in range(NSC):
            nc.sync.dma_start(out=xs_tiles[g],
                              in_=xs_re[:, g * SCH:(g + 1) * SCH, :])

        # --- broadcast b + s_o across partitions via rank-1 matmuls: the b
        # seed runs early (start=True), s_o accumulates on top ---
        for j in range(S // 512):
            nc.tensor.matmul(sob_psum[:, j * 512:(j + 1) * 512], ones_row,
                             b_row, start=True, stop=False)
        for j in range(S // 512):
            nc.tensor.matmul(sob_psum[:, j * 512:(j + 1) * 512], ones_row,
                             so_row[:, j * 512:(j + 1) * 512],
                             start=False, stop=True)

        # --- phase 1b + 2: simp half -> s_s + b, then outputs ---
        o_sb = None
        for g in range(NSC):
            xs = xs_tiles[g]
            for blk in range(SCH):
                t = g * SCH + blk
                prod = spool.tile([P, D], F32, tag="prod", name=f"ps{t}")
                nc.vector.tensor_mul(out=prod, in0=xs[:, blk, :],
                                     in1=w_bc[:, D:2 * D])
                nc.vector.tensor_reduce(
                    s_sb_mat[:, t:t + 1], prod,
                    axis=mybir.AxisListType.X, op=mybir.AluOpType.add)
            for blk in range(SCH):
                t = g * SCH + blk
                # tiles 0 and 15 ship as single 1 MiB stores so the output
                # stream starts one sigmoid (~2us) earlier; the rest pair up
                if t in (0, NT - 1):
                    o_sb = opool.tile([P, 2, S], F32, tag="osb",
                                      name=f"osingle{t}")
                    nc.scalar.activation(
                        o_sb[:, 0, :], sob_psum,
                        mybir.ActivationFunctionType.Sigmoid,
                        bias=s_sb_mat[:, t:t + 1], scale=1.0)
                    nc.sync.dma_start(out=out[t * P:(t + 1) * P, :],
                                      in_=o_sb[:, 0, :])
                    continue
                q = (t - 1) % 2
                if q == 0:
                    o_sb = opool.tile([P, 2, S], F32, tag="osb",
                                      name=f"opair{t // 2}")
                nc.scalar.activation(
                    o_sb[:, q, :], sob_psum,
                    mybir.ActivationFunctionType.Sigmoid,
                    bias=s_sb_mat[:, t:t + 1],
                    scale=1.0,
                )
                if q == 1:
                    r0 = (t - 1) * P
                    dst = out[r0:r0 + 2 * P, :].rearrange(
                        "(q p) i -> p q i", p=P)
                    nc.sync.dma_start(out=dst, in_=o_sb)


def build_program():
    nc = bacc.Bacc(
        "TRN2",
        debug=False,
        target_bir_lowering=False,
        num_devices=NCORES,
    )
    x = nc.dram_tensor("x", [2 * S, D], F32, kind="ExternalInput").ap()
    w = nc.dram_tensor("w", [1, 2 * D], F32, kind="ExternalInput").ap()
    bvec = nc.dram_tensor("bvec", [1, 1], F32, kind="ExternalInput").ap()
    out = nc.dram_tensor("out", [S, S], F32, kind="ExternalOutput").ap()
    with TileContext(nc) as tc:
        _kernel_body(tc, out, x, w, bvec)
    nc.compile()
    return nc


_PROGRAM = None


def _get_program():
    global _PROGRAM
    if _PROGRAM is None:
        _PROGRAM = build_program()
    return _PROGRAM


def make_in_maps(prop_state, W, b):
    prop = np.ascontiguousarray(np.asarray(prop_state, dtype=np.float32))
    w = np.ascontiguousarray(np.asarray(W, dtype=np.float32).reshape(1, 2 * D))
    bv = np.ascontiguousarray(np.asarray(b, dtype=np.float32).reshape(1, 1))
    assert prop.shape == (NCORES, 2 * S, D), prop.shape
    return [{"x": prop[i], "w": w, "bvec": bv} for i in range(NCORES)]


def kernel(A, prop_state, W, b, _trace=False):
    nc = _get_program()
    in_maps = make_in_maps(prop_state, W, b)
    res = bass_utils.run_bass_kernel_spmd(
        nc, in_maps, core_ids=list(range(NCORES)), trace=_trace)
    out = np.stack([res.results[i]["out"] for i in range(NCORES)], axis=0)
    if _trace:
        kernel.last_results = res
    return out



# revision 5
# speedup vs baseline: 1.2077x; 1.2077x over previous
"""Trainium2 Bass kernel for nn_AlignModel.

Computes out[b, j, i] = sigmoid(simp[b,j]·w_s + orig[b,i]·w_o + bias) where
orig/simp are the two halves of prop_state[b] ([B, 2S, D] -> [B,S,D] each),
w_o = W[0,:D], w_s = W[0,D:].

Sharding: data-parallel over batch B=8 across the 8 NeuronCores. Each core:
  in  x   [4096, 512] f32  (= prop_state[b])
  in  w   [1, 1024]   f32
  in  bvec[1, 1]      f32
  out out [2048, 2048] fp16, upcast to f32 on the host.  fp16 keeps
  rel-err ~1e-3 (sigmoid outputs, logits within +-5) while halving the
  store traffic: per-core HBM drops to 8 MiB in + 8 MiB out.

v2a structure:
  - All 32 dot products are ONE fused DVE scalar_tensor_tensor each:
    out=(x*1)*w, accum_out=sum -> dot.  ScalarE keeps only the 16 output
    sigmoids (its ~32us floor) and stays off the dot critical path.
  - s_o columns cast to fp16 on DVE per chunk and scatter straight into
    the broadcast row; PE rank-1 fp16 matmuls replicate b + s_o into PSUM.
  - Each output row-tile is ONE ScalarE op
      out_t = Sigmoid(s_o_bcast + bias_col_t)   (PSUM -> SBUF fp16).
  - Two off-critical-path rider ops measure HW behavior for the next
    iteration: a GpSimd f32->fp16 tile copy (rate probe) and a DVE
    reciprocal_approx_fast (custom-DVE opcode support probe).
"""

import numpy as np

import concourse.mybir as mybir
from concourse import bacc, bass_utils
from concourse.tile import TileContext

P = 128          # partitions
D = 512          # feature dim
S = 2048         # sents
NT = S // P      # 16 tiles per half
OCHUNKS = [1, 1, 2, 4, 4, 4]   # orig tiles per chunk
SCH = 4          # simp tiles per chunk (1 MiB)
NSC = NT // SCH
NCORES = 8
F32 = mybir.dt.float32
F16 = mybir.dt.float16


def _kernel_body(tc, out, x, w, bvec):
    nc = tc.nc
    # orig half, partition-outer: i = p*NT + n
    xo_re = x[0:S, :].rearrange("(p n) d -> p n d", n=NT)
    # simp half, partition-inner: j = n*P + p  (bias needs column layout)
    xs_re = x[S:2 * S, :].rearrange("(n p) d -> p n d", p=P)

    with (
        tc.tile_pool(name="consts", bufs=1) as cpool,
        tc.tile_pool(name="xin", bufs=1) as xpool,
        tc.tile_pool(name="scratch", bufs=4) as spool,
        tc.tile_pool(name="outbuf", bufs=4) as opool,
        tc.tile_pool(name="psum", bufs=1, space="PSUM") as ppool,
    ):
        # --- orig input stream: geometric chunks, all in flight at once ---
        xo_tiles = []
        n0 = 0
        for c, sz in enumerate(OCHUNKS):
            xo = xpool.tile([P, sz, D], F32, tag=f"xo{c}", name=f"xo{c}")
            nc.sync.dma_start(out=xo, in_=xo_re[:, n0:n0 + sz, :])
            xo_tiles.append(xo)
            n0 += sz

        # simp tiles; loads gated by a tiny DVE write (WAW dep) so the
        # transfers only start mid-phase-1a and don't starve the orig
        # stream at t=0.
        xs_tiles = [
            xpool.tile([P, SCH, D], F32, tag=f"xs{g}", name=f"xs{g}")
            for g in range(NSC)
        ]

        # w / b replicated across partitions by zero-stride DMA (SWDGE);
        # w_o first since it gates the first multiply.
        w_bc = cpool.tile([P, 2 * D], F32, tag="wbc")
        nc.gpsimd.dma_start(out=w_bc[:, 0:D],
                            in_=w[:, 0:D].broadcast_to([P, D]))
        nc.gpsimd.dma_start(out=w_bc[:, D:2 * D],
                            in_=w[:, D:2 * D].broadcast_to([P, D]))
        ones_row = cpool.tile([1, P], F16, tag="ones")
        nc.gpsimd.memset(ones_row, 1.0)

        s_o_mat = cpool.tile([P, NT], F32, tag="somat")   # s_o[p*16+n] @ [p,n]
        so16_mat = cpool.tile([P, NT], F16, tag="so16")
        s_sb_mat = cpool.tile([P, NT], F32, tag="ssmat")  # s_s, col t
        so_row = cpool.tile([1, S], F16, tag="sorow")
        b_sb = cpool.tile([1, 1], F32, tag="bsb")
        nc.sync.dma_start(out=b_sb, in_=bvec)
        b_row = cpool.tile([1, 512], F16, tag="brow")
        nc.gpsimd.memset(b_row, 0.0)
        nc.vector.tensor_scalar_add(b_row, b_row, b_sb)
        sob_psum = ppool.tile([P, S], F32, tag="sob")     # s_o on every row

        # --- rider probes (dead results, off the critical path) ---
        rider_in = spool.tile([P, D], F32, tag="rdi", name="rider_in")
        nc.vector.tensor_scalar_add(rider_in, w_bc[:, 0:D], 1.5)
        rider16 = spool.tile([P, D], F16, tag="rdo", name="rider16")
        nc.gpsimd.tensor_copy(out=rider16, in_=rider_in)     # gpsimd rate probe
        rider_rc = spool.tile([P, D], F32, tag="rdr", name="rider_rc")
        nc.vector.reciprocal_approx_fast(out=rider_rc, in_=rider_in)

        # --- phase 1a: orig half -> s_o (one fused DVE op per tile) ---
        n0 = 0
        for c, sz in enumerate(OCHUNKS):
            xo = xo_tiles[c]
            for blk in range(sz):
                t = n0 + blk
                prod = spool.tile([P, D], F32, tag="prod", name=f"po{t}")
                nc.vector.scalar_tensor_tensor(
                    out=prod, in0=xo[:, blk, :], scalar=1.0,
                    in1=w_bc[:, 0:D],
                    op0=mybir.AluOpType.mult, op1=mybir.AluOpType.mult,
                    accum_out=s_o_mat[:, t:t + 1])
                if t in (5, 7, 9, 11):
                    g = (t - 5) // 2
                    nc.vector.tensor_copy(
                        out=xs_tiles[g][0:1, 0, 0:1],
                        in_=prod[0:1, 0:1])
            # cast this chunk's s_o columns to fp16 and scatter them into
            # the broadcast row (tiny strided DMA, no transpose)
            nc.vector.tensor_copy(out=so16_mat[:, n0:n0 + sz],
                                  in_=s_o_mat[:, n0:n0 + sz])
            src = so16_mat[:, n0:n0 + sz]
            dst = so_row.rearrange("o (p n) -> o p n", n=NT)[:, :, n0:n0 + sz]
            nc.scalar.dma_start(out=dst, in_=src)
            n0 += sz

        # simp loads: queued on Sync behind the orig chunks, released by the
        # gate writes above
        for g in range(NSC):
            nc.sync.dma_start(out=xs_tiles[g],
                              in_=xs_re[:, g * SCH:(g + 1) * SCH, :])

        # --- broadcast b + s_o across partitions via rank-1 matmuls: the b
        # seed runs early (start=True), s_o accumulates on top ---
        for j in range(S // 512):
            nc.tensor.matmul(sob_psum[:, j * 512:(j + 1) * 512], ones_row,
                             b_row, start=True, stop=False)
        for j in range(S // 512):
            nc.tensor.matmul(sob_psum[:, j * 512:(j + 1) * 512], ones_row,
                             so_row[:, j * 512:(j + 1) * 512],
                             start=False, stop=True)

        # --- phase 1b + 2: simp half -> s_s, then outputs ---
        o_sb = None
        for g in range(NSC):
            xs = xs_tiles[g]
            for blk in range(SCH):
                t = g * SCH + blk
                prod = spool.tile([P, D], F32, tag="prod", name=f"ps{t}")
                nc.vector.scalar_tensor_tensor(
                    out=prod, in0=xs[:, blk, :], scalar=1.0,
                    in1=w_bc[:, D:2 * D],
                    op0=mybir.AluOpType.mult, op1=mybir.AluOpType.mult,
                    accum_out=s_sb_mat[:, t:t + 1])
            for blk in range(SCH):
                t = g * SCH + blk
                # tiles 0 and 15 ship as single 0.5 MiB stores so the output
                # stream starts one sigmoid (~2us) earlier; the rest pair up
                if t in (0, NT - 1):
                    o_sb = opool.tile([P, 2, S], F16, tag="osb",
                                      name=f"osingle{t}")
                    nc.scalar.activation(
                        o_sb[:, 0, :], sob_psum,
                        mybir.ActivationFunctionType.Sigmoid,
                        bias=s_sb_mat[:, t:t + 1], scale=1.0)
                    nc.sync.dma_start(out=out[t * P:(t + 1) * P, :],
                                      in_=o_sb[:, 0, :])
                    continue
                q = (t - 1) % 2
                if q == 0:
                    o_sb = opool.tile([P, 2, S], F16, tag="osb",
                                      name=f"opair{t // 2}")
                nc.scalar.activation(
                    o_sb[:, q, :], sob_psum,
                    mybir.ActivationFunctionType.Sigmoid,
                    bias=s_sb_mat[:, t:t + 1],
                    scale=1.0,
                )
                if q == 1:
                    r0 = (t - 1) * P
                    dst = out[r0:r0 + 2 * P, :].rearrange(
                        "(q p) i -> p q i", p=P)
                    nc.sync.dma_start(out=dst, in_=o_sb)


def build_program():
    nc = bacc.Bacc(
        "TRN2",
        debug=False,
        target_bir_lowering=False,
        num_devices=NCORES,
    )
    x = nc.dram_tensor("x", [2 * S, D], F32, kind="ExternalInput").ap()
    w = nc.dram_tensor("w", [1, 2 * D], F32, kind="ExternalInput").ap()
    bvec = nc.dram_tensor("bvec", [1, 1], F32, kind="ExternalInput").ap()
    out = nc.dram_tensor("out", [S, S], F16, kind="ExternalOutput").ap()
    with TileContext(nc) as tc:
        _kernel_body(tc, out, x, w, bvec)
    nc.compile()
    return nc


_PROGRAM = None


def _get_program():
    global _PROGRAM
    if _PROGRAM is None:
        _PROGRAM = build_program()
    return _PROGRAM


def make_in_maps(prop_state, W, b):
    prop = np.ascontiguousarray(np.asarray(prop_state, dtype=np.float32))
    w = np.ascontiguousarray(np.asarray(W, dtype=np.float32).reshape(1, 2 * D))
    bv = np.ascontiguousarray(np.asarray(b, dtype=np.float32).reshape(1, 1))
    assert prop.shape == (NCORES, 2 * S, D), prop.shape
    return [{"x": prop[i], "w": w, "bvec": bv} for i in range(NCORES)]


def kernel(A, prop_state, W, b, _trace=False):
    nc = _get_program()
    in_maps = make_in_maps(prop_state, W, b)
    res = bass_utils.run_bass_kernel_spmd(
        nc, in_maps, core_ids=list(range(NCORES)), trace=_trace)
    out = np.stack(
        [res.results[i]["out"] for i in range(NCORES)], axis=0
    ).astype(np.float32)
    if _trace:
        kernel.last_results = res
    return out


# revision 13
# speedup vs baseline: 1.2707x; 1.0522x over previous
"""Trainium2 Bass kernel for nn_AlignModel.

Computes out[b, j, i] = sigmoid(simp[b,j]·w_s + orig[b,i]·w_o + bias) where
orig/simp are the two halves of prop_state[b] ([B, 2S, D] -> [B,S,D] each),
w_o = W[0,:D], w_s = W[0,D:].

Sharding: data-parallel over batch B=8 across the 8 NeuronCores. Each core:
  in  x   [4096, 512] f32, out [2048, 2048] fp16 (host upcasts to f32;
  sigmoid outputs with logits within +-5 keep rel-err ~1e-3 in fp16),
  so per-core HBM traffic is 8 MiB in + 8 MiB out.

v3 structure (fixes the two v2a critical-path stalls):
  - BOTH halves consumed partition-inner (row j/i = n*128 + p), so a dot
    column [128,1] transposes into a contiguous 128-wide slice of the
    score row.  s_o columns go through ONE tiny PE transpose per chunk
    (vs. v2a's 2-byte-descriptor scatter DMAs that completed ~8us late).
  - Loads are split across TWO HWDGE queues (Sync + Tensor engines) --
    a single queue was measured at ~230-280 GB/s, two together >400.
  - All 32 dots are fused DVE scalar_tensor_tensor (mul + reduce in one
    1x-rate pass); ScalarE runs only the 16 output sigmoids.
  - sob PSUM = b (seeded early from b_row) + s_o broadcast, accumulated
    per-chunk by rank-1 fp16 matmuls from the transposed row slices.
  - Each output row-tile is ONE ScalarE op
      out_t = Sigmoid(sob_psum + bias_col_t)    (PSUM -> SBUF fp16).
"""

import numpy as np

import concourse.mybir as mybir
from concourse import bacc, bass_utils
from concourse.masks import make_identity
from concourse.tile import TileContext

P = 128          # partitions
D = 512          # feature dim
S = 2048         # sents
NT = S // P      # 16 tiles per half
OCHUNKS = [1, 2, 4, 4, 3, 2]   # orig tiles per chunk (small tail chunk
                               # shortens the last-dot -> sigmoid latency)
OQ = ["sync", "scalar", "sync", "scalar", "sync", "scalar"]  # issue queue
SCH = 4          # simp tiles per chunk (1 MiB)
NSC = NT // SCH
SQ = ["sync", "scalar", "sync", "scalar"]
GATE_AT = {6: 0, 8: 1, 10: 2, 12: 3}   # orig dot t -> simp group released
NCORES = 8
F32 = mybir.dt.float32
F16 = mybir.dt.float16


def _kernel_body(tc, out, x, w, bvec):
    nc = tc.nc
    # BOTH halves partition-inner: row = n*P + p
    xo_re = x[0:S, :].rearrange("(n p) d -> p n d", p=P)
    xs_re = x[S:2 * S, :].rearrange("(n p) d -> p n d", p=P)
    eng = {"sync": nc.sync, "tensor": nc.tensor, "scalar": nc.scalar}

    with (
        tc.tile_pool(name="consts", bufs=1) as cpool,
        tc.tile_pool(name="xin", bufs=1) as xpool,
        tc.tile_pool(name="scratch", bufs=4) as spool,
        tc.tile_pool(name="outbuf", bufs=4) as opool,
        tc.tile_pool(name="psum", bufs=1, space="PSUM") as ppool,
        tc.tile_pool(name="trpsum", bufs=2, space="PSUM") as tpool,
    ):
        b_sb = cpool.tile([1, 1], F32, tag="bsb")
        nc.sync.dma_start(out=b_sb, in_=bvec)

        # --- orig input stream: chunks split across two HWDGE queues ---
        xo_tiles = []
        n0 = 0
        for c, sz in enumerate(OCHUNKS):
            xo = xpool.tile([P, sz, D], F32, tag=f"xo{c}", name=f"xo{c}")
            eng[OQ[c]].dma_start(out=xo, in_=xo_re[:, n0:n0 + sz, :])
            xo_tiles.append(xo)
            n0 += sz

        # simp tiles; loads gated by a tiny DVE write (WAW dep) so the
        # transfers start mid-phase-1a without starving the orig stream.
        xs_tiles = [
            xpool.tile([P, SCH, D], F32, tag=f"xs{g}", name=f"xs{g}")
            for g in range(NSC)
        ]

        # w / b replicated across partitions by zero-stride DMA (SWDGE)
        w_bc = cpool.tile([P, 2 * D], F32, tag="wbc")
        nc.gpsimd.dma_start(out=w_bc[:, 0:D],
                            in_=w[:, 0:D].broadcast_to([P, D]))
        nc.gpsimd.dma_start(out=w_bc[:, D:2 * D],
                            in_=w[:, D:2 * D].broadcast_to([P, D]))
        ones_row = cpool.tile([1, P], F16, tag="ones")
        nc.gpsimd.memset(ones_row, 1.0)
        ident = cpool.tile([P, P], F32, tag="ident")
        make_identity(nc, ident)
        # sel_big[:, n*P:(n+1)*P] is the one-hot stationary that routes row n
        # of so16_2d into sob's free-block n (PE operands must sit at base
        # partition 0, so the block choice lives in the stationary mask).
        sel_big = cpool.tile([NT, S], F16, tag="selbig")
        nc.gpsimd.memset(sel_big, 0.0)
        sel3 = sel_big.rearrange("m (n j) -> m n j", j=P)
        nc.gpsimd.affine_select(
            out=sel3, in_=sel3, compare_op=mybir.AluOpType.not_equal,
            fill=1.0, base=0, pattern=[[-1, NT], [0, P]],
            channel_multiplier=1)

        s_o_mat = cpool.tile([P, NT], F32, tag="somat")   # s_o[n*128+p] @ [p,n]
        s_sb_mat = cpool.tile([P, NT], F32, tag="ssmat")  # s_s, col t
        b_row = cpool.tile([1, 512], F16, tag="brow")
        nc.gpsimd.memset(b_row, 0.0)
        nc.vector.tensor_scalar_add(b_row, b_row, b_sb)
        sob_psum = ppool.tile([P, S], F32, tag="sob")     # b + s_o, every row

        # seed the bias early so later accumulation MMs are start=False;
        # block tiling matches the per-tile accumulation MMs below.
        for n in range(NT):
            nc.tensor.matmul(sob_psum[:, n * P:(n + 1) * P], ones_row,
                             b_row[:, 0:P], start=True, stop=False,
                             skip_group_check=True)

        # simp loads: issued on both queues, gated (emitted here so the
        # sync/tensor sequencers reach them after the orig issues)
        for g in range(NSC):
            eng[SQ[g]].dma_start(out=xs_tiles[g],
                                 in_=xs_re[:, g * SCH:(g + 1) * SCH, :])

        # --- phase 1a: orig -> s_o columns -> transposed row slices ---
        n0 = 0
        for c, sz in enumerate(OCHUNKS):
            xo = xo_tiles[c]
            for blk in range(sz):
                t = n0 + blk
                prod = spool.tile([P, D], F32, tag="prod", name=f"po{t}")
                nc.vector.scalar_tensor_tensor(
                    out=prod, in0=xo[:, blk, :], scalar=1.0,
                    in1=w_bc[:, 0:D],
                    op0=mybir.AluOpType.mult, op1=mybir.AluOpType.mult,
                    accum_out=s_o_mat[:, t:t + 1])
                if t in GATE_AT:
                    g = GATE_AT[t]
                    nc.vector.tensor_copy(
                        out=xs_tiles[g][0:1, 0, 0:1],
                        in_=prod[0:1, 0:1])
            # transpose the chunk's s_o columns into row form (PE), cast to
            # fp16 (DVE), and accumulate into sob via rank-1 matmuls.
            tr = tpool.tile([sz, P], F32, tag="tr", name=f"tr{c}")
            nc.tensor.transpose(tr, s_o_mat[:, n0:n0 + sz], ident)
            so16_c = spool.tile([sz, P], F16, tag="so16", name=f"so16c{c}")
            nc.vector.tensor_copy(out=so16_c, in_=tr)
            for m in range(sz):
                n = n0 + m
                nc.tensor.matmul(sob_psum[:, n * P:(n + 1) * P],
                                 sel_big[0:sz, m * P:(m + 1) * P], so16_c,
                                 start=False, stop=True,
                                 skip_group_check=True)
            n0 += sz

        # --- phase 1b + 2: simp dots, then the sigmoid stream ---
        o_sb = None
        for g in range(NSC):
            xs = xs_tiles[g]
            for blk in range(SCH):
                t = g * SCH + blk
                prod = spool.tile([P, D], F32, tag="prod", name=f"ps{t}")
                nc.vector.scalar_tensor_tensor(
                    out=prod, in0=xs[:, blk, :], scalar=1.0,
                    in1=w_bc[:, D:2 * D],
                    op0=mybir.AluOpType.mult, op1=mybir.AluOpType.mult,
                    accum_out=s_sb_mat[:, t:t + 1])
            for blk in range(SCH):
                t = g * SCH + blk
                # tiles 0 and 15 ship as single 0.5 MiB stores so the output
                # stream starts one sigmoid (~2us) earlier; the rest pair up
                if t in (0, NT - 1):
                    o_sb = opool.tile([P, 2, S], F16, tag="osb",
                                      name=f"osingle{t}")
                    nc.scalar.activation(
                        o_sb[:, 0, :], sob_psum,
                        mybir.ActivationFunctionType.Sigmoid,
                        bias=s_sb_mat[:, t:t + 1], scale=1.0)
                    nc.sync.dma_start(out=out[t * P:(t + 1) * P, :],
                                      in_=o_sb[:, 0, :])
                    continue
                q = (t - 1) % 2
                if q == 0:
                    o_sb = opool.tile([P, 2, S], F16, tag="osb",
                                      name=f"opair{t // 2}")
                nc.scalar.activation(
                    o_sb[:, q, :], sob_psum,
                    mybir.ActivationFunctionType.Sigmoid,
                    bias=s_sb_mat[:, t:t + 1],
                    scale=1.0,
                )
                if q == 1:
                    r0 = (t - 1) * P
                    dst = out[r0:r0 + 2 * P, :].rearrange(
                        "(q p) i -> p q i", p=P)
                    nc.sync.dma_start(out=dst, in_=o_sb)


def build_program():
    nc = bacc.Bacc(
        "TRN2",
        debug=False,
        target_bir_lowering=False,
        num_devices=NCORES,
    )
    x = nc.dram_tensor("x", [2 * S, D], F32, kind="ExternalInput").ap()
    w = nc.dram_tensor("w", [1, 2 * D], F32, kind="ExternalInput").ap()
    bvec = nc.dram_tensor("bvec", [1, 1], F32, kind="ExternalInput").ap()
    out = nc.dram_tensor("out", [S, S], F16, kind="ExternalOutput").ap()
    with TileContext(nc) as tc:
        _kernel_body(tc, out, x, w, bvec)
    nc.compile()
    return nc


_PROGRAM = None


def _get_program():
    global _PROGRAM
    if _PROGRAM is None:
        _PROGRAM = build_program()
    return _PROGRAM


def make_in_maps(prop_state, W, b):
    prop = np.ascontiguousarray(np.asarray(prop_state, dtype=np.float32))
    w = np.ascontiguousarray(np.asarray(W, dtype=np.float32).reshape(1, 2 * D))
    bv = np.ascontiguousarray(np.asarray(b, dtype=np.float32).reshape(1, 1))
    assert prop.shape == (NCORES, 2 * S, D), prop.shape
    return [{"x": prop[i], "w": w, "bvec": bv} for i in range(NCORES)]


def kernel(A, prop_state, W, b, _trace=False):
    nc = _get_program()
    in_maps = make_in_maps(prop_state, W, b)
    res = bass_utils.run_bass_kernel_spmd(
        nc, in_maps, core_ids=list(range(NCORES)), trace=_trace)
    out = np.stack(
        [res.results[i]["out"] for i in range(NCORES)], axis=0
    ).astype(np.float32)
    if _trace:
        kernel.last_results = res
    return out


# revision 15
# speedup vs baseline: 1.3496x; 1.0621x over previous
"""Trainium2 Bass kernel for nn_AlignModel.

Computes out[b, j, i] = sigmoid(simp[b,j]·w_s + orig[b,i]·w_o + bias) where
orig/simp are the two halves of prop_state[b] ([B, 2S, D] -> [B,S,D] each),
w_o = W[0,:D], w_s = W[0,D:].

Sharding: data-parallel over batch B=8 across the 8 NeuronCores. Each core:
  in  x   [4096, 512] f32, out [2048, 2048] fp16 (host upcasts to f32;
  sigmoid outputs with logits within +-5 keep rel-err ~3e-3 in fp16),
  so per-core HBM traffic is 8 MiB in + 8 MiB out.

v3c schedule (measured-HW-driven):
  - w broadcast rides the two HWDGE queues FIRST (SWDGE landed ~12-16us
    and stalled the first dot).
  - Both halves partition-inner (row = n*128 + p; 2 KiB descriptors cap
    the two queues at ~390 GB/s aggregate -- acceptable; partition-outer
    breaks the transpose-based row formation and the bias columns).
  - simp loads are emitted AFTER the DVE gate writes so the WAW dep
    holds them back until mid-phase-1a (emitting them first inverts the
    dependency and the early simp transfers starve the orig stream --
    that was v3's 36us sigmoid start).
  - s_o chunk columns -> per-chunk PE transpose (partition-0 tiles) ->
    DVE fp16 cast -> rank-16 one-hot-stationary MMs into sob PSUM.
    Only the last chunk's ~1.5us sits on the critical path.
  - All 32 dots are fused DVE scalar_tensor_tensor; ScalarE runs only
    the 16 sigmoids (2.0us each, PSUM -> SBUF fp16).
  - Stores: pairs on Sync; tile 15 ships via GpSimd SWDGE as a
    bandwidth probe for a later store split.
"""

import numpy as np

import concourse.mybir as mybir
from concourse import bacc, bass_utils
from concourse.masks import make_identity
from concourse.tile import TileContext

P = 128          # partitions
D = 512          # feature dim
S = 2048         # sents
NT = S // P      # 16 tiles per half
OCHUNKS = [1, 2, 4, 4, 3, 2]   # orig tiles per chunk
OQ = ["sync", "scalar", "sync", "scalar", "sync", "scalar"]
SCH = 4          # simp tiles per group (1 MiB)
NSC = NT // SCH
SQ = ["sync", "scalar", "sync", "scalar"]
GATE_AT = {4: 0, 6: 1, 8: 2, 10: 3}   # orig dot t -> simp group released
NCORES = 8
F32 = mybir.dt.float32
F16 = mybir.dt.float16


def _kernel_body(tc, out, x, w, bvec):
    nc = tc.nc
    # BOTH halves partition-inner: row = n*P + p
    xo_re = x[0:S, :].rearrange("(n p) d -> p n d", p=P)
    xs_re = x[S:2 * S, :].rearrange("(n p) d -> p n d", p=P)
    eng = {"sync": nc.sync, "scalar": nc.scalar}

    with (
        tc.tile_pool(name="consts", bufs=1) as cpool,
        tc.tile_pool(name="xin", bufs=1) as xpool,
        tc.tile_pool(name="scratch", bufs=4) as spool,
        tc.tile_pool(name="outbuf", bufs=4) as opool,
        tc.tile_pool(name="psum", bufs=1, space="PSUM") as ppool,
        tc.tile_pool(name="trpsum", bufs=2, space="PSUM") as tpool,
    ):
        # weights first on the fast queues: they gate the very first dot
        w_bc = cpool.tile([P, 2 * D], F32, tag="wbc")
        nc.sync.dma_start(out=w_bc[:, 0:D],
                          in_=w[:, 0:D].broadcast_to([P, D]))
        nc.scalar.dma_start(out=w_bc[:, D:2 * D],
                            in_=w[:, D:2 * D].broadcast_to([P, D]))
        b_sb = cpool.tile([1, 1], F32, tag="bsb")
        nc.sync.dma_start(out=b_sb, in_=bvec)

        # --- orig input stream: chunks alternate between the two queues ---
        xo_tiles = []
        n0 = 0
        for c, sz in enumerate(OCHUNKS):
            xo = xpool.tile([P, sz, D], F32, tag=f"xo{c}", name=f"xo{c}")
            eng[OQ[c]].dma_start(out=xo, in_=xo_re[:, n0:n0 + sz, :])
            xo_tiles.append(xo)
            n0 += sz

        xs_tiles = [
            xpool.tile([P, SCH, D], F32, tag=f"xs{g}", name=f"xs{g}")
            for g in range(NSC)
        ]

        ones_row = cpool.tile([1, P], F16, tag="ones")
        nc.gpsimd.memset(ones_row, 1.0)
        ident = cpool.tile([P, P], F32, tag="ident")
        make_identity(nc, ident)
        # sel_big[0:sz, m*P:(m+1)*P] one-hot-selects local row m of a
        # transposed chunk into sob's free block (PE operands must sit at
        # base partition 0, so the block choice lives in the stationary).
        sel_big = cpool.tile([NT, S], F16, tag="selbig")
        nc.gpsimd.memset(sel_big, 0.0)
        sel3 = sel_big.rearrange("m (n j) -> m n j", j=P)
        nc.gpsimd.affine_select(
            out=sel3, in_=sel3, compare_op=mybir.AluOpType.not_equal,
            fill=1.0, base=0, pattern=[[-1, NT], [0, P]],
            channel_multiplier=1)

        s_o_mat = cpool.tile([P, NT], F32, tag="somat")   # s_o[n*128+p] @ [p,n]
        s_sb_mat = cpool.tile([P, NT], F32, tag="ssmat")  # s_s col t
        b_row = cpool.tile([1, P], F16, tag="brow")
        nc.gpsimd.memset(b_row, 0.0)
        nc.vector.tensor_scalar_add(b_row, b_row, b_sb)
        sob_psum = ppool.tile([P, S], F32, tag="sob")     # b + s_o, every row

        # bias seeds, same 128-col block tiling as the accumulation MMs
        for n in range(NT):
            nc.tensor.matmul(sob_psum[:, n * P:(n + 1) * P], ones_row,
                             b_row, start=True, stop=False,
                             skip_group_check=True)

        # --- phase 1a: orig dots; per-chunk transpose -> sob accumulation ---
        n0 = 0
        for c, sz in enumerate(OCHUNKS):
            xo = xo_tiles[c]
            for blk in range(sz):
                t = n0 + blk
                prod = spool.tile([P, D], F32, tag="prod", name=f"po{t}")
                nc.vector.scalar_tensor_tensor(
                    out=prod, in0=xo[:, blk, :], scalar=1.0,
                    in1=w_bc[:, 0:D],
                    op0=mybir.AluOpType.mult, op1=mybir.AluOpType.mult,
                    accum_out=s_o_mat[:, t:t + 1])
                if t in GATE_AT:
                    # WAW gate: this DVE write must be emitted BEFORE the
                    # simp dma_start below so the load waits for it.
                    g = GATE_AT[t]
                    nc.vector.tensor_copy(
                        out=xs_tiles[g][0:1, 0, 0:1],
                        in_=prod[0:1, 0:1])
            tr = tpool.tile([sz, P], F32, tag="tr", name=f"tr{c}")
            nc.tensor.transpose(tr, s_o_mat[:, n0:n0 + sz], ident)
            so16_c = spool.tile([sz, P], F16, tag="so16", name=f"so16c{c}")
            nc.vector.tensor_copy(out=so16_c, in_=tr)
            for m in range(sz):
                n = n0 + m
                nc.tensor.matmul(sob_psum[:, n * P:(n + 1) * P],
                                 sel_big[0:sz, m * P:(m + 1) * P], so16_c,
                                 start=False, stop=True,
                                 skip_group_check=True)
            n0 += sz

        # simp loads: emitted after the gate writes (WAW order) so they
        # release only mid-phase-1a; issued across both queues.
        for g in range(NSC):
            eng[SQ[g]].dma_start(out=xs_tiles[g],
                                 in_=xs_re[:, g * SCH:(g + 1) * SCH, :])

        # --- phase 1b + 2: simp dots, then the sigmoid stream ---
        o_sb = None
        for g in range(NSC):
            xs = xs_tiles[g]
            for blk in range(SCH):
                t = g * SCH + blk
                prod = spool.tile([P, D], F32, tag="prod", name=f"ps{t}")
                nc.vector.scalar_tensor_tensor(
                    out=prod, in0=xs[:, blk, :], scalar=1.0,
                    in1=w_bc[:, D:2 * D],
                    op0=mybir.AluOpType.mult, op1=mybir.AluOpType.mult,
                    accum_out=s_sb_mat[:, t:t + 1])
            for blk in range(SCH):
                t = g * SCH + blk
                # tile 0 ships single so the store stream starts early;
                # tile 15 ships single via SWDGE as a bandwidth probe.
                if t in (0, NT - 1):
                    o_sb = opool.tile([P, 2, S], F16, tag="osb",
                                      name=f"osingle{t}")
                    nc.scalar.activation(
                        o_sb[:, 0, :], sob_psum,
                        mybir.ActivationFunctionType.Sigmoid,
                        bias=s_sb_mat[:, t:t + 1], scale=1.0)
                    e = nc.gpsimd if t == NT - 1 else nc.sync
                    e.dma_start(out=out[t * P:(t + 1) * P, :],
                                in_=o_sb[:, 0, :])
                    continue
                q = (t - 1) % 2
                if q == 0:
                    o_sb = opool.tile([P, 2, S], F16, tag="osb",
                                      name=f"opair{t // 2}")
                nc.scalar.activation(
                    o_sb[:, q, :], sob_psum,
                    mybir.ActivationFunctionType.Sigmoid,
                    bias=s_sb_mat[:, t:t + 1],
                    scale=1.0,
                )
                if q == 1:
                    r0 = (t - 1) * P
                    dst = out[r0:r0 + 2 * P, :].rearrange(
                        "(q p) i -> p q i", p=P)
                    nc.sync.dma_start(out=dst, in_=o_sb)


def build_program():
    nc = bacc.Bacc(
        "TRN2",
        debug=False,
        target_bir_lowering=False,
        num_devices=NCORES,
    )
    x = nc.dram_tensor("x", [2 * S, D], F32, kind="ExternalInput").ap()
    w = nc.dram_tensor("w", [1, 2 * D], F32, kind="ExternalInput").ap()
    bvec = nc.dram_tensor("bvec", [1, 1], F32, kind="ExternalInput").ap()
    out = nc.dram_tensor("out", [S, S], F16, kind="ExternalOutput").ap()
    with TileContext(nc) as tc:
        _kernel_body(tc, out, x, w, bvec)
    nc.compile()
    return nc


_PROGRAM = None


def _get_program():
    global _PROGRAM
    if _PROGRAM is None:
        _PROGRAM = build_program()
    return _PROGRAM


def make_in_maps(prop_state, W, b):
    prop = np.ascontiguousarray(np.asarray(prop_state, dtype=np.float32))
    w = np.ascontiguousarray(np.asarray(W, dtype=np.float32).reshape(1, 2 * D))
    bv = np.ascontiguousarray(np.asarray(b, dtype=np.float32).reshape(1, 1))
    assert prop.shape == (NCORES, 2 * S, D), prop.shape
    return [{"x": prop[i], "w": w, "bvec": bv} for i in range(NCORES)]


def kernel(A, prop_state, W, b, _trace=False):
    nc = _get_program()
    in_maps = make_in_maps(prop_state, W, b)
    res = bass_utils.run_bass_kernel_spmd(
        nc, in_maps, core_ids=list(range(NCORES)), trace=_trace)
    out = np.stack(
        [res.results[i]["out"] for i in range(NCORES)], axis=0
    ).astype(np.float32)
    if _trace:
        kernel.last_results = res
    return out


# revision 16
# speedup vs baseline: 1.3761x; 1.0196x over previous
"""Trainium2 Bass kernel for nn_AlignModel.

Computes out[b, j, i] = sigmoid(simp[b,j]·w_s + orig[b,i]·w_o + bias) where
orig/simp are the two halves of prop_state[b] ([B, 2S, D] -> [B,S,D] each),
w_o = W[0,:D], w_s = W[0,D:].

Sharding: data-parallel over batch B=8 across the 8 NeuronCores. Each core:
  in  x   [4096, 512] f32, out [2048, 2048] fp16 (host upcasts to f32;
  sigmoid outputs with logits within +-5 keep rel-err ~3e-3 in fp16),
  so per-core HBM traffic is 8 MiB in + 8 MiB out.

v3c schedule (measured-HW-driven):
  - w broadcast rides the two HWDGE queues FIRST (SWDGE landed ~12-16us
    and stalled the first dot).
  - Both halves partition-inner (row = n*128 + p; 2 KiB descriptors cap
    the two queues at ~390 GB/s aggregate -- acceptable; partition-outer
    breaks the transpose-based row formation and the bias columns).
  - simp loads are emitted AFTER the DVE gate writes so the WAW dep
    holds them back until mid-phase-1a (emitting them first inverts the
    dependency and the early simp transfers starve the orig stream --
    that was v3's 36us sigmoid start).
  - s_o chunk columns -> per-chunk PE transpose (partition-0 tiles) ->
    DVE fp16 cast -> rank-16 one-hot-stationary MMs into sob PSUM.
    Only the last chunk's ~1.5us sits on the critical path.
  - All 32 dots are fused DVE scalar_tensor_tensor; ScalarE runs only
    the 16 sigmoids (2.0us each, PSUM -> SBUF fp16).
  - Stores: pairs on Sync; tile 15 ships via GpSimd SWDGE as a
    bandwidth probe for a later store split.
"""

import numpy as np

import concourse.mybir as mybir
from concourse import bacc, bass_utils
from concourse.masks import make_identity
from concourse.tile import TileContext

P = 128          # partitions
D = 512          # feature dim
S = 2048         # sents
NT = S // P      # 16 tiles per half
OCHUNKS = [1, 2, 4, 4, 3, 2]   # orig tiles per chunk
OQ = ["sync", "scalar", "sync", "scalar", "sync", "scalar"]
SCH = 4          # simp tiles per group (1 MiB)
NSC = NT // SCH
SQ = ["sync", "scalar", "sync", "scalar"]
GATE_AT = {4: 0, 6: 1, 8: 2, 10: 3}   # orig dot t -> simp group released
NCORES = 8
F32 = mybir.dt.float32
F16 = mybir.dt.float16


def _kernel_body(tc, out, x, w, bvec):
    nc = tc.nc
    # BOTH halves partition-inner: row = n*P + p
    xo_re = x[0:S, :].rearrange("(n p) d -> p n d", p=P)
    xs_re = x[S:2 * S, :].rearrange("(n p) d -> p n d", p=P)
    eng = {"sync": nc.sync, "scalar": nc.scalar}

    with (
        tc.tile_pool(name="consts", bufs=1) as cpool,
        tc.tile_pool(name="xin", bufs=1) as xpool,
        tc.tile_pool(name="scratch", bufs=4) as spool,
        tc.tile_pool(name="outbuf", bufs=4) as opool,
        tc.tile_pool(name="psum", bufs=1, space="PSUM") as ppool,
        tc.tile_pool(name="trpsum", bufs=2, space="PSUM") as tpool,
    ):
        # w arrives host-replicated [P, 2D]: one fast 4 KiB-descriptor DMA
        # per half (an on-device zero-stride broadcast DMA serialized on a
        # single DRAM page at ~40 GB/s and delayed the first dot by ~6us).
        w_bc = cpool.tile([P, 2 * D], F32, tag="wbc")
        nc.sync.dma_start(out=w_bc[:, 0:D], in_=w[:, 0:D])
        nc.scalar.dma_start(out=w_bc[:, D:2 * D], in_=w[:, D:2 * D])
        b_row = cpool.tile([1, P], F16, tag="brow")
        nc.sync.dma_start(out=b_row, in_=bvec)

        # --- orig input stream: chunks alternate between the two queues ---
        xo_tiles = []
        n0 = 0
        for c, sz in enumerate(OCHUNKS):
            xo = xpool.tile([P, sz, D], F32, tag=f"xo{c}", name=f"xo{c}")
            eng[OQ[c]].dma_start(out=xo, in_=xo_re[:, n0:n0 + sz, :])
            xo_tiles.append(xo)
            n0 += sz

        xs_tiles = [
            xpool.tile([P, SCH, D], F32, tag=f"xs{g}", name=f"xs{g}")
            for g in range(NSC)
        ]

        ones_row = cpool.tile([1, P], F16, tag="ones")
        nc.gpsimd.memset(ones_row, 1.0)
        ident = cpool.tile([P, P], F32, tag="ident")
        make_identity(nc, ident)
        # sel_big[0:sz, m*P:(m+1)*P] one-hot-selects local row m of a
        # transposed chunk into sob's free block (PE operands must sit at
        # base partition 0, so the block choice lives in the stationary).
        sel_big = cpool.tile([NT, S], F16, tag="selbig")
        nc.gpsimd.memset(sel_big, 0.0)
        sel3 = sel_big.rearrange("m (n j) -> m n j", j=P)
        nc.gpsimd.affine_select(
            out=sel3, in_=sel3, compare_op=mybir.AluOpType.not_equal,
            fill=1.0, base=0, pattern=[[-1, NT], [0, P]],
            channel_multiplier=1)

        s_o_mat = cpool.tile([P, NT], F32, tag="somat")   # s_o[n*128+p] @ [p,n]
        s_sb_mat = cpool.tile([P, NT], F32, tag="ssmat")  # s_s col t
        sob_psum = ppool.tile([P, S], F32, tag="sob")     # b + s_o, every row

        # bias seeds, same 128-col block tiling as the accumulation MMs
        for n in range(NT):
            nc.tensor.matmul(sob_psum[:, n * P:(n + 1) * P], ones_row,
                             b_row, start=True, stop=False,
                             skip_group_check=True)

        # --- phase 1a: orig dots; per-chunk transpose -> sob accumulation ---
        n0 = 0
        for c, sz in enumerate(OCHUNKS):
            xo = xo_tiles[c]
            for blk in range(sz):
                t = n0 + blk
                prod = spool.tile([P, D], F32, tag="prod", name=f"po{t}")
                nc.vector.scalar_tensor_tensor(
                    out=prod, in0=xo[:, blk, :], scalar=1.0,
                    in1=w_bc[:, 0:D],
                    op0=mybir.AluOpType.mult, op1=mybir.AluOpType.mult,
                    accum_out=s_o_mat[:, t:t + 1])
                if t in GATE_AT:
                    # WAW gate: this DVE write must be emitted BEFORE the
                    # simp dma_start below so the load waits for it.
                    g = GATE_AT[t]
                    nc.vector.tensor_copy(
                        out=xs_tiles[g][0:1, 0, 0:1],
                        in_=prod[0:1, 0:1])
            tr = tpool.tile([sz, P], F32, tag="tr", name=f"tr{c}")
            nc.tensor.transpose(tr, s_o_mat[:, n0:n0 + sz], ident)
            so16_c = spool.tile([sz, P], F16, tag="so16", name=f"so16c{c}")
            nc.vector.tensor_copy(out=so16_c, in_=tr)
            for m in range(sz):
                n = n0 + m
                nc.tensor.matmul(sob_psum[:, n * P:(n + 1) * P],
                                 sel_big[0:sz, m * P:(m + 1) * P], so16_c,
                                 start=False, stop=True,
                                 skip_group_check=True)
            n0 += sz

        # simp loads: emitted after the gate writes (WAW order) so they
        # release only mid-phase-1a; issued across both queues.
        for g in range(NSC):
            eng[SQ[g]].dma_start(out=xs_tiles[g],
                                 in_=xs_re[:, g * SCH:(g + 1) * SCH, :])

        # --- phase 1b + 2: simp dots, then the sigmoid stream ---
        o_sb = None
        for g in range(NSC):
            xs = xs_tiles[g]
            for blk in range(SCH):
                t = g * SCH + blk
                prod = spool.tile([P, D], F32, tag="prod", name=f"ps{t}")
                nc.vector.scalar_tensor_tensor(
                    out=prod, in0=xs[:, blk, :], scalar=1.0,
                    in1=w_bc[:, D:2 * D],
                    op0=mybir.AluOpType.mult, op1=mybir.AluOpType.mult,
                    accum_out=s_sb_mat[:, t:t + 1])
            for blk in range(SCH):
                t = g * SCH + blk
                # tile 0 ships single so the store stream starts early;
                # tile 15 ships single via SWDGE as a bandwidth probe.
                if t in (0, NT - 1):
                    o_sb = opool.tile([P, 2, S], F16, tag="osb",
                                      name=f"osingle{t}")
                    nc.scalar.activation(
                        o_sb[:, 0, :], sob_psum,
                        mybir.ActivationFunctionType.Sigmoid,
                        bias=s_sb_mat[:, t:t + 1], scale=1.0)
                    e = nc.gpsimd if t == NT - 1 else nc.sync
                    e.dma_start(out=out[t * P:(t + 1) * P, :],
                                in_=o_sb[:, 0, :])
                    continue
                q = (t - 1) % 2
                if q == 0:
                    o_sb = opool.tile([P, 2, S], F16, tag="osb",
                                      name=f"opair{t // 2}")
                nc.scalar.activation(
                    o_sb[:, q, :], sob_psum,
                    mybir.ActivationFunctionType.Sigmoid,
                    bias=s_sb_mat[:, t:t + 1],
                    scale=1.0,
                )
                if q == 1:
                    r0 = (t - 1) * P
                    dst = out[r0:r0 + 2 * P, :].rearrange(
                        "(q p) i -> p q i", p=P)
                    nc.sync.dma_start(out=dst, in_=o_sb)


def build_program():
    nc = bacc.Bacc(
        "TRN2",
        debug=False,
        target_bir_lowering=False,
        num_devices=NCORES,
    )
    x = nc.dram_tensor("x", [2 * S, D], F32, kind="ExternalInput").ap()
    w = nc.dram_tensor("w", [P, 2 * D], F32, kind="ExternalInput").ap()
    bvec = nc.dram_tensor("bvec", [1, P], mybir.dt.float16,
                          kind="ExternalInput").ap()
    out = nc.dram_tensor("out", [S, S], F16, kind="ExternalOutput").ap()
    with TileContext(nc) as tc:
        _kernel_body(tc, out, x, w, bvec)
    nc.compile()
    return nc


_PROGRAM = None


def _get_program():
    global _PROGRAM
    if _PROGRAM is None:
        _PROGRAM = build_program()
    return _PROGRAM


def make_in_maps(prop_state, W, b):
    prop = np.ascontiguousarray(np.asarray(prop_state, dtype=np.float32))
    w = np.ascontiguousarray(np.broadcast_to(
        np.asarray(W, dtype=np.float32).reshape(1, 2 * D), (P, 2 * D)))
    bv = np.ascontiguousarray(np.broadcast_to(
        np.asarray(b, dtype=np.float16).reshape(1, 1), (1, P)))
    assert prop.shape == (NCORES, 2 * S, D), prop.shape
    return [{"x": prop[i], "w": w, "bvec": bv} for i in range(NCORES)]


def kernel(A, prop_state, W, b, _trace=False):
    nc = _get_program()
    in_maps = make_in_maps(prop_state, W, b)
    res = bass_utils.run_bass_kernel_spmd(
        nc, in_maps, core_ids=list(range(NCORES)), trace=_trace)
    out = np.stack(
        [res.results[i]["out"] for i in range(NCORES)], axis=0
    ).astype(np.float32)
    if _trace:
        kernel.last_results = res
    return out
